# revision 8
# baseline (speedup 1.0000x reference)
"""CorrLookup Trainium2 kernel (8 NeuronCores, SPMD data-parallel over pixels).

Reference op: for each pixel n (N = B*H*W = 16384) and each pyramid level l,
bilinear-sample an 81-point (9x9, radius 4) window centered at
(x_n + flow_x)/2^l from that pixel's own (H_l, W_l) correlation map, with
zero padding outside the map. Output (B, 4*81, H, W) f32.

Strategy per core (2048 pixels, pixel-per-partition, 16 groups of 128):
  - Host ships each level's maps TRANSPOSED (x-major), fp16, zero-padded, so
    a window's footprint is one contiguous span of 9*H_l+10 halfs.
  - On-chip: per-pixel span starts + bilinear weights + edge masks from flow
    (DVE, level-stacked [128,64] ops), one batched indirect DMA per level
    (GpSimd SWDGE, offset AP [128,16]), then a separable fp16 bilinear mix
    per level with packed (2x_1p) operands throughout; the y-mix result is
    written transposed so the x-mix also keeps packed weight operands.
  - Weights fold the zero-pad masks; garbage read by edge spans is masked.
"""

import os
import sys
import types
import numpy as np

B, H, W = 2, 64, 128
N = B * H * W
N_CORES = 8
NPX = N // N_CORES  # 2048
GPP = NPX // 128  # 16 pixel groups ("waves") per partition
R = 4
K = 2 * R + 1  # 9
LV = [(64, 128), (32, 64), (16, 32), (8, 16)]  # (Hc, Wc) per level
NL = 4
PAD = 4096
SHIFT = 64.0  # coordinate shift so truncation sees positive values
LAST_EXEC_NS = None

_prog = None


def _install_trace_shim():
    try:
        import antenv

        if "antenv.axon_hooks" not in sys.modules:
            mod = types.ModuleType("antenv.axon_hooks")
            _h = [None]
            mod.set_axon_ntff_profile_hook = lambda hk: _h.__setitem__(0, hk)
            mod.get_axon_ntff_profile_hook = lambda: _h[0]
            sys.modules["antenv.axon_hooks"] = mod
            antenv.axon_hooks = mod
        from antenv.axon_hooks import set_axon_ntff_profile_hook

        from trn_agent_boot.trn_boot import _ntff_profile_via_ctypes

        set_axon_ntff_profile_hook(
            _ntff_profile_via_ctypes("/opt/axon/libaxon_pjrt.so")
        )
        import concourse.bass_utils as bu

        bu.upload_artifacts = lambda tmpdir: f"file://{tmpdir}"
        return True
    except Exception:
        return False


def _build():
    import concourse.bacc as bacc
    import concourse.bass as bass
    import concourse.tile as tile
    import concourse.mybir as mybir

    f32 = mybir.dt.float32
    f16 = mybir.dt.float16
    i32 = mybir.dt.int32
    Alu = mybir.AluOpType

    nc = bacc.Bacc("TRN2", target_bir_lowering=False, debug=False, num_devices=N_CORES)

    # two half-tensors per level: SWDGE loses element-offset precision past
    # 2^24 elements (fp32 path in descriptor gen), so keep offsets < 2^24
    srcs = []
    for l, (Hc, Wc) in enumerate(LV):
        tot = (NPX // 2) * Hc * Wc + 2 * PAD
        srcs.append([
            nc.dram_tensor(f"src{l}{h}", [tot, 1], f16, kind="ExternalInput").ap()
            for h in "ab"
        ])
    flx = nc.dram_tensor("flx", [128, GPP], f32, kind="ExternalInput").ap()
    fly = nc.dram_tensor("fly", [128, GPP], f32, kind="ExternalInput").ap()
    bxc = nc.dram_tensor("bx", [128, GPP], f32, kind="ExternalInput").ap()
    byc = nc.dram_tensor("by", [128, GPP], f32, kind="ExternalInput").ap()
    svecd = nc.dram_tensor("svec", [128, NL], f32, kind="ExternalInput").ap()
    hvecd = nc.dram_tensor("hvec", [128, NL], f32, kind="ExternalInput").ap()
    based = nc.dram_tensor("base", [128, NL * GPP], i32, kind="ExternalInput").ap()
    iod = nc.dram_tensor("iot", [128, 10], f16, kind="ExternalInput").ap()
    ubxd = nc.dram_tensor("ubx", [128, NL * GPP * 10], f16, kind="ExternalInput").ap()
    ubyd = nc.dram_tensor("uby", [128, NL * GPP * 10], f16, kind="ExternalInput").ap()
    outs = [
        nc.dram_tensor(f"out{l}", [128, GPP * 81], f16, kind="ExternalOutput").ap()
        for l in range(NL)
    ]

    LG = NL * GPP  # 64 stacked (level, group) scalars per partition
    LGR = LG * 10  # 640

    def AP(tile_ap, off_extra, dims):
        # dims: list of [step, count] for free axes; partition dim copied
        base = tile_ap
        return bass.AP(base.tensor, base.offset + off_extra, [list(base.ap[0])] + dims)

    with tile.TileContext(nc) as tc:
        with (
            tc.tile_pool(name="const", bufs=1) as cp,
            tc.tile_pool(name="patch", bufs=1) as pp,
            tc.tile_pool(name="work", bufs=1) as wp,
        ):
            # ---- load constants / flow (sync + scalar engines in parallel) ----
            flx_t = cp.tile([128, GPP], f32)
            fly_t = cp.tile([128, GPP], f32)
            bx_t = cp.tile([128, GPP], f32)
            by_t = cp.tile([128, GPP], f32)
            sv_t = cp.tile([128, NL], f32)
            hv_t = cp.tile([128, NL], f32)
            base_t = cp.tile([128, LG], i32)
            io_t = cp.tile([128, 10], f16)
            ubx_t = cp.tile([128, LGR], f16)
            uby_t = cp.tile([128, LGR], f16)
            nc.sync.dma_start(out=flx_t[:], in_=flx)
            nc.sync.dma_start(out=fly_t[:], in_=fly)
            nc.sync.dma_start(out=bx_t[:], in_=bxc)
            nc.sync.dma_start(out=by_t[:], in_=byc)
            nc.sync.dma_start(out=sv_t[:], in_=svecd)
            nc.sync.dma_start(out=hv_t[:], in_=hvecd)
            nc.scalar.dma_start(out=base_t[:], in_=based)
            nc.scalar.dma_start(out=io_t[:], in_=iod)
            nc.scalar.dma_start(out=ubx_t[:], in_=ubxd)
            nc.scalar.dma_start(out=uby_t[:], in_=ubyd)

            # ---- stage A: per-(level, pixel-group) scalars, f32 [128, 64] ----
            gx = wp.tile([128, GPP], f32, tag="gx")
            gy = wp.tile([128, GPP], f32, tag="gy")
            nc.vector.tensor_tensor(out=gx[:], in0=bx_t[:], in1=flx_t[:], op=Alu.add)
            nc.vector.tensor_tensor(out=gy[:], in0=by_t[:], in1=fly_t[:], op=Alu.add)

            # cx = gx * s_l + SHIFT (positive); broadcast gx over levels
            cx = wp.tile([128, LG], f32, tag="cx")
            cy = wp.tile([128, LG], f32, tag="cy")
            gx_b = AP(gx[:], 0, [[0, NL], [1, GPP]])
            gy_b = AP(gy[:], 0, [[0, NL], [1, GPP]])
            sv_b = AP(sv_t[:], 0, [[1, NL], [0, GPP]])
            cx_v = AP(cx[:], 0, [[GPP, NL], [1, GPP]])
            cy_v = AP(cy[:], 0, [[GPP, NL], [1, GPP]])
            nc.vector.tensor_tensor(out=cx_v, in0=gx_b, in1=sv_b, op=Alu.mult)
            nc.vector.tensor_tensor(out=cy_v, in0=gy_b, in1=sv_b, op=Alu.mult)
            nc.vector.tensor_scalar(
                out=cx[:], in0=cx[:], scalar1=SHIFT, scalar2=None, op0=Alu.add
            )
            nc.vector.tensor_scalar(
                out=cy[:], in0=cy[:], scalar1=SHIFT, scalar2=None, op0=Alu.add
            )

            # floor via int cast (rounding-mode independent fix-up)
            wx = wp.tile([128, LG], f32, tag="wx")
            wy = wp.tile([128, LG], f32, tag="wy")
            fx = wp.tile([128, LG], f32, tag="fx")
            fy = wp.tile([128, LG], f32, tag="fy")
            for c_t, w_t, f_t, sfx in ((cx, wx, fx, "x"), (cy, wy, fy, "y")):
                fi = wp.tile([128, LG], i32, tag=f"fi{sfx}")
                ff = wp.tile([128, LG], f32, tag=f"ff{sfx}")
                dd = wp.tile([128, LG], f32, tag=f"dd{sfx}")
                mm = wp.tile([128, LG], f32, tag=f"mm{sfx}")
                nc.vector.tensor_copy(out=fi[:], in_=c_t[:])
                nc.vector.tensor_copy(out=ff[:], in_=fi[:])
                nc.vector.tensor_tensor(out=dd[:], in0=c_t[:], in1=ff[:], op=Alu.subtract)
                nc.vector.tensor_scalar(out=mm[:], in0=dd[:], scalar1=0.0, scalar2=None, op0=Alu.is_lt)
                nc.vector.tensor_tensor(out=w_t[:], in0=dd[:], in1=mm[:], op=Alu.add)
                nc.vector.tensor_tensor(out=f_t[:], in0=ff[:], in1=mm[:], op=Alu.subtract)

            # ---- span start index = (fx-S-4)*Hc + (fy-S-4) + n*Hc*Wc + PAD ----
            # host folds PAD + n*Hc*Wc - (SHIFT+4)*(Hc+1) into base
            idxf = wp.tile([128, LG], f32, tag="idxf")
            hv_b = AP(hv_t[:], 0, [[1, NL], [0, GPP]])
            idxf_v = AP(idxf[:], 0, [[GPP, NL], [1, GPP]])
            nc.vector.tensor_tensor(out=idxf_v, in0=AP(fx[:], 0, [[GPP, NL], [1, GPP]]), in1=hv_b, op=Alu.mult)
            nc.vector.tensor_tensor(out=idxf[:], in0=idxf[:], in1=fy[:], op=Alu.add)
            nc.vector.tensor_scalar(
                out=idxf[:], in0=idxf[:], scalar1=0.25, scalar2=None, op0=Alu.add
            )
            idxi = wp.tile([128, LG], i32, tag="idxi")
            nc.vector.tensor_copy(out=idxi[:], in_=idxf[:])
            idx = wp.tile([128, LG], i32, tag="idx")
            nc.vector.tensor_tensor(out=idx[:], in0=idxi[:], in1=base_t[:], op=Alu.add)

            # ---- gathers: per-wave indirect DMA (ucode: 1 offset/partition) ----
            # order smallest level first so its mix can start earliest
            patches = {}
            for l in (3, 2, 1, 0):
                Hc, Wc = LV[l]
                span = 9 * Hc + 10
                patch = pp.tile([128, GPP * span], f16, tag=f"patch{l}")
                patches[l] = patch
                for w in range(GPP):
                    nc.gpsimd.indirect_dma_start(
                        out=patch[:, w * span : (w + 1) * span],
                        out_offset=None,
                        in_=srcs[l][0 if w < GPP // 2 else 1],
                        in_offset=bass.IndirectOffsetOnAxis(
                            ap=idx[:, l * GPP + w : l * GPP + w + 1], axis=0
                        ),
                    )

            # ---- masks & folded weights, fp16, level-stacked ----
            omx = wp.tile([128, LG], f32, tag="omx")
            omy = wp.tile([128, LG], f32, tag="omy")
            nc.vector.tensor_scalar(
                out=omx[:], in0=wx[:], scalar1=-1.0, scalar2=1.0, op0=Alu.mult, op1=Alu.add
            )
            nc.vector.tensor_scalar(
                out=omy[:], in0=wy[:], scalar1=-1.0, scalar2=1.0, op0=Alu.mult, op1=Alu.add
            )
            fx16 = wp.tile([128, LG], f16, tag="fx16")
            fy16 = wp.tile([128, LG], f16, tag="fy16")
            nc.vector.tensor_copy(out=fx16[:], in_=fx[:])
            nc.vector.tensor_copy(out=fy16[:], in_=fy[:])

            # xs[p,(l,g),r] = fx - SHIFT - 4 + r  (true x coord of column r)
            xs = wp.tile([128, LGR], f16, tag="xs")
            ys = wp.tile([128, LGR], f16, tag="ys")
            io_b = AP(io_t[:], 0, [[0, LG], [1, 10]])
            nc.vector.tensor_tensor(
                out=xs[:], in0=AP(fx16[:], 0, [[1, LG], [0, 10]]), in1=io_b, op=Alu.add
            )
            nc.vector.tensor_tensor(
                out=ys[:], in0=AP(fy16[:], 0, [[1, LG], [0, 10]]), in1=io_b, op=Alu.add
            )
            # clip to [0, Wc-1]/[0, Hc-1]; mask = (clipped == raw)
            xc = wp.tile([128, LGR], f16, tag="xc")
            yc = wp.tile([128, LGR], f16, tag="yc")
            nc.vector.tensor_tensor(out=xc[:], in0=xs[:], in1=ubx_t[:], op=Alu.min)
            nc.vector.tensor_tensor(out=yc[:], in0=ys[:], in1=uby_t[:], op=Alu.min)
            nc.vector.tensor_scalar(out=xc[:], in0=xc[:], scalar1=0.0, scalar2=None, op0=Alu.max)
            nc.vector.tensor_scalar(out=yc[:], in0=yc[:], scalar1=0.0, scalar2=None, op0=Alu.max)
            cmx = wp.tile([128, LGR], f16, tag="cmx")
            cmy = wp.tile([128, LGR], f16, tag="cmy")
            nc.vector.tensor_tensor(out=cmx[:], in0=xc[:], in1=xs[:], op=Alu.is_equal)
            nc.vector.tensor_tensor(out=cmy[:], in0=yc[:], in1=ys[:], op=Alu.is_equal)

            # broadcast-materialize the scalar weights to 9 taps (cast f32->f16)
            LG9 = LG * 9  # 576
            omyr = wp.tile([128, LG9], f16, tag="omyr")
            wyr = wp.tile([128, LG9], f16, tag="wyr")
            omxr = wp.tile([128, LG9], f16, tag="omxr")
            wxr = wp.tile([128, LG9], f16, tag="wxr")
            nc.vector.tensor_copy(out=omyr[:], in_=AP(omy[:], 0, [[1, LG], [0, 9]]))
            nc.vector.tensor_copy(out=wyr[:], in_=AP(wy[:], 0, [[1, LG], [0, 9]]))
            nc.vector.tensor_copy(out=omxr[:], in_=AP(omx[:], 0, [[1, LG], [0, 9]]))
            nc.vector.tensor_copy(out=wxr[:], in_=AP(wx[:], 0, [[1, LG], [0, 9]]))
            # w0[(l,g),b] = (1-wy)*cmy[b], w1 = wy*cmy[b+1] (b = y tap)
            # v0[(l,g),a] = (1-wx)*cmx[a], v1 = wx*cmx[a+1] (a = x tap)
            w0 = wp.tile([128, LG9], f16, tag="w0")
            w1 = wp.tile([128, LG9], f16, tag="w1")
            v0 = wp.tile([128, LG9], f16, tag="v0")
            v1 = wp.tile([128, LG9], f16, tag="v1")
            cmy0 = AP(cmy[:], 0, [[10, LG], [1, 9]])
            cmy1 = AP(cmy[:], 1, [[10, LG], [1, 9]])
            cmx0 = AP(cmx[:], 0, [[10, LG], [1, 9]])
            cmx1 = AP(cmx[:], 1, [[10, LG], [1, 9]])
            nc.vector.tensor_tensor(out=w0[:], in0=cmy0, in1=omyr[:], op=Alu.mult)
            nc.vector.tensor_tensor(out=w1[:], in0=cmy1, in1=wyr[:], op=Alu.mult)
            nc.vector.tensor_tensor(out=v0[:], in0=cmx0, in1=omxr[:], op=Alu.mult)
            nc.vector.tensor_tensor(out=v1[:], in0=cmx1, in1=wxr[:], op=Alu.mult)

            # ---- bilinear mix per level (fp16, packed operands) ----
            # L3..L1 whole; L0 in two wave-halves so the last mix chunk only
            # trails the final 8 gathers.
            def mix(l, w_lo, n_w, ot):
                Hc, Wc = LV[l]
                span = 9 * Hc + 10
                patch = patches[l]
                po = w_lo * span
                # P view [128, g, a(10 cols, stride Hc), b(9 rows, stride 1)]
                P0 = AP(patch[:], po, [[span, n_w], [Hc, 10], [1, 9]])
                P1 = AP(patch[:], po + 1, [[span, n_w], [Hc, 10], [1, 9]])
                t1 = wp.tile([128, n_w * 90], f16, tag=f"t1_{n_w}")
                t2 = wp.tile([128, n_w * 90], f16, tag=f"t2_{n_w}")
                t1v = AP(t1[:], 0, [[90, n_w], [9, 10], [1, 9]])
                t2v = AP(t2[:], 0, [[90, n_w], [9, 10], [1, 9]])
                wo = l * GPP * 9 + w_lo * 9
                w0b = AP(w0[:], wo, [[9, n_w], [0, 10], [1, 9]])
                w1b = AP(w1[:], wo, [[9, n_w], [0, 10], [1, 9]])
                # y-mix: t[g,a,b] = P[g,a,b]*w0[b] ; P[g,a,b+1]*w1[b]
                nc.vector.tensor_tensor(out=t1v, in0=P0, in1=w0b, op=Alu.mult)
                nc.vector.tensor_tensor(out=t2v, in0=P1, in1=w1b, op=Alu.mult)
                # qbT[g,b,a] = t1[g,a,b] + t2[g,a,b]  (transposed write)
                qb = wp.tile([128, n_w * 90], f16, tag=f"qb_{n_w}")
                qbT = AP(qb[:], 0, [[90, n_w], [10, 9], [1, 10]])
                t1t = AP(t1[:], 0, [[90, n_w], [1, 9], [9, 10]])
                t2t = AP(t2[:], 0, [[90, n_w], [1, 9], [9, 10]])
                nc.vector.tensor_tensor(out=qbT, in0=t1t, in1=t2t, op=Alu.add)
                # x-mix: out[g,b,a] = qb[g,b,a]*v0[a] + qb[g,b,a+1]*v1[a]
                u1 = wp.tile([128, n_w * 81], f16, tag=f"u1_{n_w}")
                u2 = wp.tile([128, n_w * 81], f16, tag=f"u2_{n_w}")
                Qa0 = AP(qb[:], 0, [[90, n_w], [10, 9], [1, 9]])
                Qa1 = AP(qb[:], 1, [[90, n_w], [10, 9], [1, 9]])
                u1v = AP(u1[:], 0, [[81, n_w], [9, 9], [1, 9]])
                u2v = AP(u2[:], 0, [[81, n_w], [9, 9], [1, 9]])
                v0b = AP(v0[:], wo, [[9, n_w], [0, 9], [1, 9]])
                v1b = AP(v1[:], wo, [[9, n_w], [0, 9], [1, 9]])
                nc.vector.tensor_tensor(out=u1v, in0=Qa0, in1=v0b, op=Alu.mult)
                nc.vector.tensor_tensor(out=u2v, in0=Qa1, in1=v1b, op=Alu.mult)
                oslice = ot[:, w_lo * 81 : (w_lo + n_w) * 81]
                nc.vector.tensor_tensor(out=oslice, in0=u1[:], in1=u2[:], op=Alu.add)

            for l in (3, 2, 1):
                ot = wp.tile([128, GPP * 81], f16, tag=f"ot{l}")
                mix(l, 0, GPP, ot)
                nc.sync.dma_start(out=outs[l], in_=ot[:])
            ot0 = wp.tile([128, GPP * 81], f16, tag="ot0")
            half = GPP // 2
            mix(0, 0, half, ot0)
            mix(0, half, half, ot0)
            nc.sync.dma_start(out=outs[0], in_=ot0[:])

    nc.compile()
    return nc


def _marshal(corr0, corr1, corr2, corr3, flow):
    """Build per-core input maps."""
    corrs = [corr0, corr1, corr2, corr3]
    fl = np.ascontiguousarray(flow.transpose(0, 2, 3, 1).reshape(N, 2))
    wgrid = np.tile(np.arange(W, dtype=np.float32), H * B)
    hgrid = np.tile(np.repeat(np.arange(H, dtype=np.float32), W), B)
    iota = np.tile(
        (np.arange(10, dtype=np.float16) - np.float16(4.0 + SHIFT)).reshape(1, 10),
        (128, 1),
    )
    svec = np.tile(
        np.array([1.0 / (1 << l) for l in range(NL)], dtype=np.float32), (128, 1)
    )
    hvec = np.tile(
        np.array([float(LV[l][0]) for l in range(NL)], dtype=np.float32), (128, 1)
    )
    # upper bounds per (l, g, r): Wc-1 / Hc-1 constant per level
    ubx = np.tile(
        np.repeat(
            np.array([LV[l][1] - 1 for l in range(NL)], dtype=np.float16), GPP * 10
        ).reshape(1, -1),
        (128, 1),
    )
    uby = np.tile(
        np.repeat(
            np.array([LV[l][0] - 1 for l in range(NL)], dtype=np.float16), GPP * 10
        ).reshape(1, -1),
        (128, 1),
    )
    # base[(p),(l,g)] = PAD + n_half*Hc*Wc - (SHIFT+4)*(Hc+1);
    # n_half = (g mod 8)*128 + p (waves 0-7 -> half a, 8-15 -> half b)
    p_ar = np.arange(128)[:, None, None]
    g_ar = np.arange(GPP)[None, None, :]
    base = np.empty((128, NL, GPP), dtype=np.int64)
    for l, (Hc, Wc) in enumerate(LV):
        base[:, l, :] = (
            PAD
            + ((g_ar % (GPP // 2)) * 128 + p_ar) * (Hc * Wc)
            - int((SHIFT + 4) * (Hc + 1))
        )[:, 0, :]
    base = base.reshape(128, NL * GPP).astype(np.int32)

    in_maps = []
    for c in range(N_CORES):
        m = {}
        lo = c * NPX
        for l, (Hc, Wc) in enumerate(LV):
            shard = corrs[l].reshape(N, Hc, Wc)[lo : lo + NPX]
            tr = np.ascontiguousarray(shard.transpose(0, 2, 1)).reshape(NPX, -1)
            half = NPX // 2
            for h, sl in (("a", slice(0, half)), ("b", slice(half, NPX))):
                buf = np.zeros(half * Hc * Wc + 2 * PAD, dtype=np.float16)
                buf[PAD : PAD + half * Hc * Wc] = tr[sl].reshape(-1)
                m[f"src{l}{h}"] = buf.reshape(-1, 1)
        wm = lambda a: np.ascontiguousarray(a.reshape(GPP, 128).T)
        m["flx"] = wm(fl[lo : lo + NPX, 0])
        m["fly"] = wm(fl[lo : lo + NPX, 1])
        m["bx"] = wm(wgrid[lo : lo + NPX])
        m["by"] = wm(hgrid[lo : lo + NPX])
        m["svec"] = svec
        m["hvec"] = hvec
        m["base"] = base
        m["iot"] = iota
        m["ubx"] = ubx
        m["uby"] = uby
        in_maps.append(m)
    return in_maps


def kernel(corr0, corr1, corr2, corr3, flow):
    global _prog, LAST_EXEC_NS
    trace = os.environ.get("CORR_TRACE") == "1"
    if trace:
        trace = _install_trace_shim()
    from concourse.bass_utils import run_bass_kernel_spmd

    if _prog is None:
        _prog = _build()
    in_maps = _marshal(corr0, corr1, corr2, corr3, flow)
    res = run_bass_kernel_spmd(
        _prog,
        in_maps,
        core_ids=list(range(N_CORES)),
        trace=trace,
        trace_cores=[0] if trace else None,
    )
    LAST_EXEC_NS = res.exec_time_ns
    if trace and res.instructions_and_trace:
        kernel.last_insts = res.instructions_and_trace
    # assemble: out[n, l*81 + a*9 + b]; device gives [p, g, b, a]
    full = np.empty((N, 324), dtype=np.float32)
    for c in range(N_CORES):
        lo = c * NPX
        for l in range(4):
            o = res.results[c][f"out{l}"].reshape(128, GPP, 9, 9).astype(np.float32)
            full[lo : lo + NPX, l * 81 : (l + 1) * 81] = (
                o.transpose(1, 0, 3, 2).reshape(NPX, 81)
            )
    return np.ascontiguousarray(
        full.reshape(B, H, W, 324).transpose(0, 3, 1, 2)
    )


# revision 16
# speedup vs baseline: 1.0456x; 1.0456x over previous
"""CorrLookup Trainium2 kernel (8 NeuronCores, SPMD data-parallel over pixels).

Reference op: for each pixel n (N = B*H*W = 16384) and each pyramid level l,
bilinear-sample an 81-point (9x9, radius 4) window centered at
(x_n + flow_x)/2^l from that pixel's own (H_l, W_l) correlation map, with
zero padding outside the map. Output (B, 4*81, H, W) f32.

Strategy per core (2048 pixels, pixel-per-partition, 16 groups of 128):
  - Host ships each level's maps TRANSPOSED (x-major), fp16, zero-padded, so
    a window's footprint is one contiguous span of 9*H_l+10 halfs.
  - On-chip: per-pixel span starts + bilinear weights + edge masks from flow
    (DVE, level-stacked [128,64] ops), one batched indirect DMA per level
    (GpSimd SWDGE, offset AP [128,16]), then a separable fp16 bilinear mix
    per level with packed (2x_1p) operands throughout; the y-mix result is
    written transposed so the x-mix also keeps packed weight operands.
  - Weights fold the zero-pad masks; garbage read by edge spans is masked.
"""

import os
import sys
import types
import numpy as np

B, H, W = 2, 64, 128
N = B * H * W
N_CORES = 8
NPX = N // N_CORES  # 2048
GPP = NPX // 128  # 16 pixel groups ("waves") per partition
R = 4
K = 2 * R + 1  # 9
LV = [(64, 128), (32, 64), (16, 32), (8, 16)]  # (Hc, Wc) per level
NL = 4
PAD = 4096
SHIFT = 64.0  # coordinate shift so truncation sees positive values
LAST_EXEC_NS = None

_prog = None


def _install_trace_shim():
    try:
        import antenv

        if "antenv.axon_hooks" not in sys.modules:
            mod = types.ModuleType("antenv.axon_hooks")
            _h = [None]
            mod.set_axon_ntff_profile_hook = lambda hk: _h.__setitem__(0, hk)
            mod.get_axon_ntff_profile_hook = lambda: _h[0]
            sys.modules["antenv.axon_hooks"] = mod
            antenv.axon_hooks = mod
        from antenv.axon_hooks import set_axon_ntff_profile_hook

        from trn_agent_boot.trn_boot import _ntff_profile_via_ctypes

        set_axon_ntff_profile_hook(
            _ntff_profile_via_ctypes("/opt/axon/libaxon_pjrt.so")
        )
        import concourse.bass_utils as bu

        bu.upload_artifacts = lambda tmpdir: f"file://{tmpdir}"
        return True
    except Exception:
        return False


def _build():
    import concourse.bacc as bacc
    import concourse.bass as bass
    import concourse.tile as tile
    import concourse.mybir as mybir

    f32 = mybir.dt.float32
    f16 = mybir.dt.float16
    i32 = mybir.dt.int32
    Alu = mybir.AluOpType

    nc = bacc.Bacc("TRN2", target_bir_lowering=False, debug=False, num_devices=N_CORES)

    # two half-tensors per level: SWDGE loses element-offset precision past
    # 2^24 elements (fp32 path in descriptor gen), so keep offsets < 2^24
    srcs = []
    for l, (Hc, Wc) in enumerate(LV):
        tot = (NPX // 2) * Hc * Wc + 2 * PAD
        srcs.append([
            nc.dram_tensor(f"src{l}{h}", [tot, 1], f16, kind="ExternalInput").ap()
            for h in "ab"
        ])
    flxy = nc.dram_tensor("flxy", [128, 2 * GPP], f32, kind="ExternalInput").ap()
    svecd = nc.dram_tensor("svec", [128, NL], f32, kind="ExternalInput").ap()
    # bxys: per-level prescaled base coords (bx*s+SHIFT | by*s+SHIFT)
    bxys = nc.dram_tensor("bxys", [128, 2 * NL * GPP], f32, kind="ExternalInput").ap()
    # base cols 0..63: folded span-start base; cols 64..67: Hc per level (i32)
    based = nc.dram_tensor("base", [128, NL * GPP + NL], i32, kind="ExternalInput").ap()
    iod = nc.dram_tensor("iot", [128, 10], f16, kind="ExternalInput").ap()
    ubxd = nc.dram_tensor("ubx", [128, NL * GPP * 10], f16, kind="ExternalInput").ap()
    ubyd = nc.dram_tensor("uby", [128, NL * GPP * 10], f16, kind="ExternalInput").ap()
    outs = [
        nc.dram_tensor(f"out{l}", [128, GPP * 81], f16, kind="ExternalOutput").ap()
        for l in range(NL)
    ]

    LG = NL * GPP  # 64 stacked (level, group) scalars per partition
    LGR = LG * 10  # 640

    def AP(tile_ap, off_extra, dims):
        # dims: list of [step, count] for free axes; partition dim copied
        base = tile_ap
        return bass.AP(base.tensor, base.offset + off_extra, [list(base.ap[0])] + dims)

    with tile.TileContext(nc) as tc:
        with (
            tc.tile_pool(name="const", bufs=1) as cp,
            tc.tile_pool(name="patch", bufs=1) as pp,
            tc.tile_pool(name="work", bufs=1) as wp,
        ):
            # ---- load constants / flow (sync + scalar engines in parallel) ----
            flxy_t = cp.tile([128, 2 * GPP], f32)
            sv_t = cp.tile([128, NL], f32)
            bxys_t = cp.tile([128, 2 * LG], f32)
            base_t = cp.tile([128, LG + NL], i32)
            io_t = cp.tile([128, 10], f16)
            ubx_t = cp.tile([128, LGR], f16)
            uby_t = cp.tile([128, LGR], f16)
            # idx-critical tensors first on both engines
            nc.sync.dma_start(out=flxy_t[:], in_=flxy)
            nc.sync.dma_start(out=sv_t[:], in_=svecd)
            nc.sync.dma_start(out=bxys_t[:], in_=bxys)
            nc.scalar.dma_start(out=base_t[:], in_=based)
            nc.scalar.dma_start(out=io_t[:], in_=iod)
            nc.scalar.dma_start(out=ubx_t[:], in_=ubxd)
            nc.scalar.dma_start(out=uby_t[:], in_=ubyd)

            # ---- stage A: cx = bxs + flx*s_l, level-stacked [128, 64] f32 ----
            cx = wp.tile([128, LG], f32, tag="cx")
            cy = wp.tile([128, LG], f32, tag="cy")
            sv_b = AP(sv_t[:], 0, [[1, NL], [0, GPP]])
            flx_b = AP(flxy_t[:], 0, [[0, NL], [1, GPP]])
            fly_b = AP(flxy_t[:], GPP, [[0, NL], [1, GPP]])
            cx_v = AP(cx[:], 0, [[GPP, NL], [1, GPP]])
            cy_v = AP(cy[:], 0, [[GPP, NL], [1, GPP]])
            nc.vector.tensor_tensor(out=cx_v, in0=flx_b, in1=sv_b, op=Alu.mult)
            nc.vector.tensor_tensor(out=cy_v, in0=fly_b, in1=sv_b, op=Alu.mult)
            nc.vector.tensor_tensor(out=cx[:], in0=cx[:], in1=bxys_t[:, :LG], op=Alu.add)
            nc.vector.tensor_tensor(out=cy[:], in0=cy[:], in1=bxys_t[:, LG:], op=Alu.add)

            # floor via int cast (round-nearest; fix-up) -> f32 floor + i32 floor
            wx = wp.tile([128, LG], f32, tag="wx")
            wy = wp.tile([128, LG], f32, tag="wy")
            fx = wp.tile([128, LG], f32, tag="fx")
            fy = wp.tile([128, LG], f32, tag="fy")
            fxi = wp.tile([128, LG], i32, tag="fxi")
            fyi = wp.tile([128, LG], i32, tag="fyi")
            for c_t, w_t, f_t, fi_t, sfx in (
                (cx, wx, fx, fxi, "x"),
                (cy, wy, fy, fyi, "y"),
            ):
                fi = wp.tile([128, LG], i32, tag=f"fi{sfx}")
                ff = wp.tile([128, LG], f32, tag=f"ff{sfx}")
                dd = wp.tile([128, LG], f32, tag=f"dd{sfx}")
                mm = wp.tile([128, LG], f32, tag=f"mm{sfx}")
                nc.vector.tensor_copy(out=fi[:], in_=c_t[:])
                nc.vector.tensor_copy(out=ff[:], in_=fi[:])
                nc.vector.tensor_tensor(out=dd[:], in0=c_t[:], in1=ff[:], op=Alu.subtract)
                nc.vector.tensor_scalar(out=mm[:], in0=dd[:], scalar1=0.0, scalar2=None, op0=Alu.is_lt)
                nc.vector.tensor_tensor(out=w_t[:], in0=dd[:], in1=mm[:], op=Alu.add)
                nc.vector.tensor_tensor(out=f_t[:], in0=ff[:], in1=mm[:], op=Alu.subtract)
                nc.vector.tensor_copy(out=fi_t[:], in_=f_t[:])

            # ---- span start index on GpSimd (int): fx*Hc + fy + base ----
            # host folds PAD + n*Hc*Wc - (SHIFT+4)*(Hc+1) into base
            hvi_b = AP(base_t[:], LG, [[1, NL], [0, GPP]])
            idxt = wp.tile([128, LG], i32, tag="idxt")
            nc.gpsimd.tensor_tensor(out=idxt[:], in0=fxi[:], in1=hvi_b, op=Alu.mult)
            nc.gpsimd.tensor_tensor(out=idxt[:], in0=idxt[:], in1=fyi[:], op=Alu.add)
            idx = wp.tile([128, LG], i32, tag="idx")
            nc.gpsimd.tensor_tensor(out=idx[:], in0=idxt[:], in1=base_t[:, :LG], op=Alu.add)

            # ---- gathers: per-wave indirect DMA (ucode: 1 offset/partition) ----
            # order smallest level first so its mix can start earliest
            patches = {}
            for l in (3, 2, 1, 0):
                Hc, Wc = LV[l]
                span = 9 * Hc + 10
                patch = pp.tile([128, GPP * span], f16, tag=f"patch{l}")
                patches[l] = patch
                for w in range(GPP):
                    nc.gpsimd.indirect_dma_start(
                        out=patch[:, w * span : (w + 1) * span],
                        out_offset=None,
                        in_=srcs[l][0 if w < GPP // 2 else 1],
                        in_offset=bass.IndirectOffsetOnAxis(
                            ap=idx[:, l * GPP + w : l * GPP + w + 1], axis=0
                        ),
                    )

            # ---- masks & folded weights, fp16, level-stacked ----
            omx = wp.tile([128, LG], f32, tag="omx")
            omy = wp.tile([128, LG], f32, tag="omy")
            nc.vector.tensor_scalar(
                out=omx[:], in0=wx[:], scalar1=-1.0, scalar2=1.0, op0=Alu.mult, op1=Alu.add
            )
            nc.vector.tensor_scalar(
                out=omy[:], in0=wy[:], scalar1=-1.0, scalar2=1.0, op0=Alu.mult, op1=Alu.add
            )
            fx16 = wp.tile([128, LG], f16, tag="fx16")
            fy16 = wp.tile([128, LG], f16, tag="fy16")
            nc.vector.tensor_copy(out=fx16[:], in_=fx[:])
            nc.vector.tensor_copy(out=fy16[:], in_=fy[:])

            # xs[p,(l,g),r] = fx - SHIFT - 4 + r  (true x coord of column r)
            xs = wp.tile([128, LGR], f16, tag="xs")
            ys = wp.tile([128, LGR], f16, tag="ys")
            io_b = AP(io_t[:], 0, [[0, LG], [1, 10]])
            nc.vector.tensor_tensor(
                out=xs[:], in0=AP(fx16[:], 0, [[1, LG], [0, 10]]), in1=io_b, op=Alu.add
            )
            nc.vector.tensor_tensor(
                out=ys[:], in0=AP(fy16[:], 0, [[1, LG], [0, 10]]), in1=io_b, op=Alu.add
            )
            # clip to [0, Wc-1]/[0, Hc-1]; mask = (clipped == raw)
            xc = wp.tile([128, LGR], f16, tag="xc")
            yc = wp.tile([128, LGR], f16, tag="yc")
            nc.vector.tensor_tensor(out=xc[:], in0=xs[:], in1=ubx_t[:], op=Alu.min)
            nc.vector.tensor_tensor(out=yc[:], in0=ys[:], in1=uby_t[:], op=Alu.min)
            nc.vector.tensor_scalar(out=xc[:], in0=xc[:], scalar1=0.0, scalar2=None, op0=Alu.max)
            nc.vector.tensor_scalar(out=yc[:], in0=yc[:], scalar1=0.0, scalar2=None, op0=Alu.max)
            cmx = wp.tile([128, LGR], f16, tag="cmx")
            cmy = wp.tile([128, LGR], f16, tag="cmy")
            nc.vector.tensor_tensor(out=cmx[:], in0=xc[:], in1=xs[:], op=Alu.is_equal)
            nc.vector.tensor_tensor(out=cmy[:], in0=yc[:], in1=ys[:], op=Alu.is_equal)

            # broadcast-materialize the scalar weights to 9 taps (cast f32->f16)
            LG9 = LG * 9  # 576
            omyr = wp.tile([128, LG9], f16, tag="omyr")
            wyr = wp.tile([128, LG9], f16, tag="wyr")
            omxr = wp.tile([128, LG9], f16, tag="omxr")
            wxr = wp.tile([128, LG9], f16, tag="wxr")
            nc.vector.tensor_copy(out=omyr[:], in_=AP(omy[:], 0, [[1, LG], [0, 9]]))
            nc.vector.tensor_copy(out=wyr[:], in_=AP(wy[:], 0, [[1, LG], [0, 9]]))
            nc.vector.tensor_copy(out=omxr[:], in_=AP(omx[:], 0, [[1, LG], [0, 9]]))
            nc.vector.tensor_copy(out=wxr[:], in_=AP(wx[:], 0, [[1, LG], [0, 9]]))
            # w0[(l,g),b] = (1-wy)*cmy[b], w1 = wy*cmy[b+1] (b = y tap)
            # v0[(l,g),a] = (1-wx)*cmx[a], v1 = wx*cmx[a+1] (a = x tap)
            w0 = wp.tile([128, LG9], f16, tag="w0")
            w1 = wp.tile([128, LG9], f16, tag="w1")
            v0 = wp.tile([128, LG9], f16, tag="v0")
            v1 = wp.tile([128, LG9], f16, tag="v1")
            cmy0 = AP(cmy[:], 0, [[10, LG], [1, 9]])
            cmy1 = AP(cmy[:], 1, [[10, LG], [1, 9]])
            cmx0 = AP(cmx[:], 0, [[10, LG], [1, 9]])
            cmx1 = AP(cmx[:], 1, [[10, LG], [1, 9]])
            nc.vector.tensor_tensor(out=w0[:], in0=cmy0, in1=omyr[:], op=Alu.mult)
            nc.vector.tensor_tensor(out=w1[:], in0=cmy1, in1=wyr[:], op=Alu.mult)
            nc.vector.tensor_tensor(out=v0[:], in0=cmx0, in1=omxr[:], op=Alu.mult)
            nc.vector.tensor_tensor(out=v1[:], in0=cmx1, in1=wxr[:], op=Alu.mult)

            # ---- bilinear mix per level (fp16, packed operands) ----
            # L3..L1 whole; L0 in two wave-halves so the last mix chunk only
            # trails the final 8 gathers.
            def mix(l, w_lo, n_w, ot):
                Hc, Wc = LV[l]
                span = 9 * Hc + 10
                patch = patches[l]
                po = w_lo * span
                # P view [128, g, a(10 cols, stride Hc), b(9 rows, stride 1)]
                P0 = AP(patch[:], po, [[span, n_w], [Hc, 10], [1, 9]])
                P1 = AP(patch[:], po + 1, [[span, n_w], [Hc, 10], [1, 9]])
                t1 = wp.tile([128, n_w * 90], f16, tag=f"t1_{n_w}")
                t2 = wp.tile([128, n_w * 90], f16, tag=f"t2_{n_w}")
                t1v = AP(t1[:], 0, [[90, n_w], [9, 10], [1, 9]])
                t2v = AP(t2[:], 0, [[90, n_w], [9, 10], [1, 9]])
                wo = l * GPP * 9 + w_lo * 9
                w0b = AP(w0[:], wo, [[9, n_w], [0, 10], [1, 9]])
                w1b = AP(w1[:], wo, [[9, n_w], [0, 10], [1, 9]])
                # y-mix: t[g,a,b] = P[g,a,b]*w0[b] ; P[g,a,b+1]*w1[b]
                nc.vector.tensor_tensor(out=t1v, in0=P0, in1=w0b, op=Alu.mult)
                nc.vector.tensor_tensor(out=t2v, in0=P1, in1=w1b, op=Alu.mult)
                # qbT[g,b,a] = t1[g,a,b] + t2[g,a,b]  (transposed write)
                qb = wp.tile([128, n_w * 90], f16, tag=f"qb_{n_w}")
                qbT = AP(qb[:], 0, [[90, n_w], [10, 9], [1, 10]])
                t1t = AP(t1[:], 0, [[90, n_w], [1, 9], [9, 10]])
                t2t = AP(t2[:], 0, [[90, n_w], [1, 9], [9, 10]])
                nc.vector.tensor_tensor(out=qbT, in0=t1t, in1=t2t, op=Alu.add)
                # x-mix: out[g,b,a] = qb[g,b,a]*v0[a] + qb[g,b,a+1]*v1[a]
                u1 = wp.tile([128, n_w * 81], f16, tag=f"u1_{n_w}")
                u2 = wp.tile([128, n_w * 81], f16, tag=f"u2_{n_w}")
                Qa0 = AP(qb[:], 0, [[90, n_w], [10, 9], [1, 9]])
                Qa1 = AP(qb[:], 1, [[90, n_w], [10, 9], [1, 9]])
                u1v = AP(u1[:], 0, [[81, n_w], [9, 9], [1, 9]])
                u2v = AP(u2[:], 0, [[81, n_w], [9, 9], [1, 9]])
                v0b = AP(v0[:], wo, [[9, n_w], [0, 9], [1, 9]])
                v1b = AP(v1[:], wo, [[9, n_w], [0, 9], [1, 9]])
                nc.vector.tensor_tensor(out=u1v, in0=Qa0, in1=v0b, op=Alu.mult)
                nc.vector.tensor_tensor(out=u2v, in0=Qa1, in1=v1b, op=Alu.mult)
                oslice = ot[:, w_lo * 81 : (w_lo + n_w) * 81]
                nc.vector.tensor_tensor(out=oslice, in0=u1[:], in1=u2[:], op=Alu.add)

            for l in (3, 2, 1):
                ot = wp.tile([128, GPP * 81], f16, tag=f"ot{l}")
                mix(l, 0, GPP, ot)
                nc.sync.dma_start(out=outs[l], in_=ot[:])
            ot0 = wp.tile([128, GPP * 81], f16, tag="ot0")
            half = GPP // 2
            mix(0, 0, half, ot0)
            nc.sync.dma_start(out=outs[0][:, : half * 81], in_=ot0[:, : half * 81])
            mix(0, half, half, ot0)
            nc.sync.dma_start(
                out=outs[0][:, half * 81 :], in_=ot0[:, half * 81 :]
            )

    nc.compile()
    return nc


def _marshal(corr0, corr1, corr2, corr3, flow):
    """Build per-core input maps."""
    corrs = [corr0, corr1, corr2, corr3]
    fl = np.ascontiguousarray(flow.transpose(0, 2, 3, 1).reshape(N, 2))
    wgrid = np.tile(np.arange(W, dtype=np.float32), H * B)
    hgrid = np.tile(np.repeat(np.arange(H, dtype=np.float32), W), B)
    iota = np.tile(
        (np.arange(10, dtype=np.float16) - np.float16(4.0 + SHIFT)).reshape(1, 10),
        (128, 1),
    )
    svec = np.tile(
        np.array([1.0 / (1 << l) for l in range(NL)], dtype=np.float32), (128, 1)
    )
    # upper bounds per (l, g, r): Wc-1 / Hc-1 constant per level
    ubx = np.tile(
        np.repeat(
            np.array([LV[l][1] - 1 for l in range(NL)], dtype=np.float16), GPP * 10
        ).reshape(1, -1),
        (128, 1),
    )
    uby = np.tile(
        np.repeat(
            np.array([LV[l][0] - 1 for l in range(NL)], dtype=np.float16), GPP * 10
        ).reshape(1, -1),
        (128, 1),
    )
    # base[(p),(l,g)] = PAD + n_half*Hc*Wc - (SHIFT+4)*(Hc+1);
    # n_half = (g mod 8)*128 + p (waves 0-7 -> half a, 8-15 -> half b)
    p_ar = np.arange(128)[:, None, None]
    g_ar = np.arange(GPP)[None, None, :]
    base = np.empty((128, NL, GPP), dtype=np.int64)
    for l, (Hc, Wc) in enumerate(LV):
        base[:, l, :] = (
            PAD
            + ((g_ar % (GPP // 2)) * 128 + p_ar) * (Hc * Wc)
            - int((SHIFT + 4) * (Hc + 1))
        )[:, 0, :]
    base = np.concatenate(
        [
            base.reshape(128, NL * GPP),
            np.tile(np.array([LV[l][0] for l in range(NL)]), (128, 1)),
        ],
        axis=1,
    ).astype(np.int32)
    # per-level prescaled base coords, level-stacked (l, g)
    bxs = np.empty((128, NL, GPP), dtype=np.float32)
    bys = np.empty((128, NL, GPP), dtype=np.float32)

    in_maps = []
    for c in range(N_CORES):
        m = {}
        lo = c * NPX
        for l, (Hc, Wc) in enumerate(LV):
            shard = corrs[l].reshape(N, Hc, Wc)[lo : lo + NPX]
            tr = np.ascontiguousarray(shard.transpose(0, 2, 1)).reshape(NPX, -1)
            half = NPX // 2
            for h, sl in (("a", slice(0, half)), ("b", slice(half, NPX))):
                buf = np.zeros(half * Hc * Wc + 2 * PAD, dtype=np.float16)
                buf[PAD : PAD + half * Hc * Wc] = tr[sl].reshape(-1)
                m[f"src{l}{h}"] = buf.reshape(-1, 1)
        wm = lambda a: np.ascontiguousarray(a.reshape(GPP, 128).T)
        m["flxy"] = np.concatenate(
            [wm(fl[lo : lo + NPX, 0]), wm(fl[lo : lo + NPX, 1])], axis=1
        )
        bxw = wm(wgrid[lo : lo + NPX])
        byw = wm(hgrid[lo : lo + NPX])
        for l in range(NL):
            s = np.float32(1.0 / (1 << l))
            bxs[:, l, :] = bxw * s + np.float32(SHIFT)
            bys[:, l, :] = byw * s + np.float32(SHIFT)
        m["bxys"] = np.concatenate(
            [bxs.reshape(128, NL * GPP), bys.reshape(128, NL * GPP)], axis=1
        ).astype(np.float32)
        m["svec"] = svec
        m["base"] = base
        m["iot"] = iota
        m["ubx"] = ubx
        m["uby"] = uby
        in_maps.append(m)
    return in_maps


def kernel(corr0, corr1, corr2, corr3, flow):
    global _prog, LAST_EXEC_NS
    trace = os.environ.get("CORR_TRACE") == "1"
    if trace:
        trace = _install_trace_shim()
    from concourse.bass_utils import run_bass_kernel_spmd

    if _prog is None:
        _prog = _build()
    in_maps = _marshal(corr0, corr1, corr2, corr3, flow)
    res = run_bass_kernel_spmd(
        _prog,
        in_maps,
        core_ids=list(range(N_CORES)),
        trace=trace,
        trace_cores=[0] if trace else None,
    )
    LAST_EXEC_NS = res.exec_time_ns
    if trace and res.instructions_and_trace:
        kernel.last_insts = res.instructions_and_trace
    # assemble: out[n, l*81 + a*9 + b]; device gives [p, g, b, a]
    full = np.empty((N, 324), dtype=np.float32)
    for c in range(N_CORES):
        lo = c * NPX
        for l in range(4):
            o = res.results[c][f"out{l}"].reshape(128, GPP, 9, 9).astype(np.float32)
            full[lo : lo + NPX, l * 81 : (l + 1) * 81] = (
                o.transpose(1, 0, 3, 2).reshape(NPX, 81)
            )
    return np.ascontiguousarray(
        full.reshape(B, H, W, 324).transpose(0, 3, 1, 2)
    )


# revision 26
# speedup vs baseline: 1.2625x; 1.2075x over previous
"""CorrLookup Trainium2 kernel (8 NeuronCores, SPMD data-parallel over pixels).

Reference op: for each pixel n (N = B*H*W = 16384) and each pyramid level l,
bilinear-sample an 81-point (9x9, radius 4) window centered at
(x_n + flow_x)/2^l from that pixel's own (H_l, W_l) correlation map, with
zero padding outside the map. Output (B, 4*81, H, W) f32.

Strategy per core (2048 pixels, pixel-per-partition, 16 groups of 128):
  - Host ships each level's maps TRANSPOSED (x-major), fp16, zero-padded, so
    a window's footprint is one contiguous span of 9*H_l+10 halfs.
  - On-chip: per-pixel span starts + bilinear weights + edge masks from flow
    (DVE, level-stacked [128,64] ops), one batched indirect DMA per level
    (GpSimd SWDGE, offset AP [128,16]), then a separable fp16 bilinear mix
    per level with packed (2x_1p) operands throughout; the y-mix result is
    written transposed so the x-mix also keeps packed weight operands.
  - Weights fold the zero-pad masks; garbage read by edge spans is masked.
"""

import os
import sys
import types
import numpy as np

B, H, W = 2, 64, 128
N = B * H * W
N_CORES = 8
NPX = N // N_CORES  # 2048
GPP = NPX // 128  # 16 pixel groups ("waves") per partition
R = 4
K = 2 * R + 1  # 9
LV = [(64, 128), (32, 64), (16, 32), (8, 16)]  # (Hc, Wc) per level
NL = 4
PAD = 4096
SHIFT = 64.0  # coordinate shift so truncation sees positive values
LAST_EXEC_NS = None

_prog = None


def _install_trace_shim():
    try:
        import antenv

        if "antenv.axon_hooks" not in sys.modules:
            mod = types.ModuleType("antenv.axon_hooks")
            _h = [None]
            mod.set_axon_ntff_profile_hook = lambda hk: _h.__setitem__(0, hk)
            mod.get_axon_ntff_profile_hook = lambda: _h[0]
            sys.modules["antenv.axon_hooks"] = mod
            antenv.axon_hooks = mod
        from antenv.axon_hooks import set_axon_ntff_profile_hook

        from trn_agent_boot.trn_boot import _ntff_profile_via_ctypes

        set_axon_ntff_profile_hook(
            _ntff_profile_via_ctypes("/opt/axon/libaxon_pjrt.so")
        )
        import concourse.bass_utils as bu

        bu.upload_artifacts = lambda tmpdir: f"file://{tmpdir}"
        return True
    except Exception:
        return False


def _build():
    import concourse.bacc as bacc
    import concourse.bass as bass
    import concourse.tile as tile
    import concourse.mybir as mybir

    f32 = mybir.dt.float32
    f16 = mybir.dt.float16
    i32 = mybir.dt.int32
    Alu = mybir.AluOpType

    nc = bacc.Bacc("TRN2", target_bir_lowering=False, debug=False, num_devices=N_CORES)

    # two half-tensors per level: SWDGE loses element-offset precision past
    # 2^24 elements (fp32 path in descriptor gen), so keep offsets < 2^24
    srcs = []
    for l, (Hc, Wc) in enumerate(LV):
        tot = (NPX // 2) * Hc * Wc + 2 * PAD
        srcs.append([
            nc.dram_tensor(f"src{l}{h}", [tot, 1], f16, kind="ExternalInput").ap()
            for h in "ab"
        ])
    flxy = nc.dram_tensor("flxy", [128, 2 * GPP], f32, kind="ExternalInput").ap()
    svecd = nc.dram_tensor("svec", [128, NL], f32, kind="ExternalInput").ap()
    # bxys: per-level prescaled base coords (bx*s+SHIFT | by*s+SHIFT)
    bxys = nc.dram_tensor("bxys", [128, 2 * NL * GPP], f32, kind="ExternalInput").ap()
    # base cols 0..63: folded span-start base; cols 64..67: Hc per level (i32)
    based = nc.dram_tensor("base", [128, NL * GPP + NL], i32, kind="ExternalInput").ap()
    iod = nc.dram_tensor("iot", [128, 10], f16, kind="ExternalInput").ap()
    ubxd = nc.dram_tensor("ubx", [128, NL * GPP * 10], f16, kind="ExternalInput").ap()
    ubyd = nc.dram_tensor("uby", [128, NL * GPP * 10], f16, kind="ExternalInput").ap()
    outs = [
        nc.dram_tensor(f"out{l}", [128, GPP * 81], f16, kind="ExternalOutput").ap()
        for l in range(NL)
    ]

    LG = NL * GPP  # 64 stacked (level, group) scalars per partition
    LGR = LG * 10  # 640

    def AP(tile_ap, off_extra, dims):
        # dims: list of [step, count] for free axes; partition dim copied
        base = tile_ap
        return bass.AP(base.tensor, base.offset + off_extra, [list(base.ap[0])] + dims)

    with tile.TileContext(nc) as tc:
        with (
            tc.tile_pool(name="const", bufs=1) as cp,
            tc.tile_pool(name="patch", bufs=1) as pp,
            tc.tile_pool(name="work", bufs=1) as wp,
        ):
            # ---- load constants / flow (sync + scalar engines in parallel) ----
            flxy_t = cp.tile([128, 2 * GPP], f32)
            sv_t = cp.tile([128, NL], f32)
            bxys_t = cp.tile([128, 2 * LG], f32)
            base_t = cp.tile([128, LG + NL], i32)
            io_t = cp.tile([128, 10], f16)
            ubx_t = cp.tile([128, LGR], f16)
            uby_t = cp.tile([128, LGR], f16)
            # idx-critical tensors first on both engines
            nc.sync.dma_start(out=flxy_t[:], in_=flxy)
            nc.sync.dma_start(out=sv_t[:], in_=svecd)
            nc.sync.dma_start(out=bxys_t[:], in_=bxys)
            nc.scalar.dma_start(out=base_t[:], in_=based)
            nc.scalar.dma_start(out=io_t[:], in_=iod)
            nc.scalar.dma_start(out=ubx_t[:], in_=ubxd)
            nc.scalar.dma_start(out=uby_t[:], in_=ubyd)

            # ---- stage A: cx = bxs + flx*s_l, level-stacked [128, 64] f32 ----
            cx = wp.tile([128, LG], f32, tag="cx")
            cy = wp.tile([128, LG], f32, tag="cy")
            sv_b = AP(sv_t[:], 0, [[1, NL], [0, GPP]])
            flx_b = AP(flxy_t[:], 0, [[0, NL], [1, GPP]])
            fly_b = AP(flxy_t[:], GPP, [[0, NL], [1, GPP]])
            cx_v = AP(cx[:], 0, [[GPP, NL], [1, GPP]])
            cy_v = AP(cy[:], 0, [[GPP, NL], [1, GPP]])
            nc.vector.tensor_tensor(out=cx_v, in0=flx_b, in1=sv_b, op=Alu.mult)
            nc.vector.tensor_tensor(out=cy_v, in0=fly_b, in1=sv_b, op=Alu.mult)
            nc.vector.tensor_tensor(out=cx[:], in0=cx[:], in1=bxys_t[:, :LG], op=Alu.add)
            nc.vector.tensor_tensor(out=cy[:], in0=cy[:], in1=bxys_t[:, LG:], op=Alu.add)

            # floor(v) = round_nearest(v - 0.49999997): a flip at cell
            # boundaries only shifts the coordinate by ~3e-8 consistently
            # (weights + idx + masks all derive from the same floor), and the
            # interpolant is continuous across cells -> error ~1e-5, way under
            # fp16 noise. 4 ops/axis instead of 7 on the gather-critical path.
            wx = wp.tile([128, LG], f32, tag="wx")
            wy = wp.tile([128, LG], f32, tag="wy")
            fx = wp.tile([128, LG], f32, tag="fx")
            fy = wp.tile([128, LG], f32, tag="fy")
            fxi = wp.tile([128, LG], i32, tag="fxi")
            fyi = wp.tile([128, LG], i32, tag="fyi")
            for c_t, w_t, f_t, fi_t, sfx in (
                (cx, wx, fx, fxi, "x"),
                (cy, wy, fy, fyi, "y"),
            ):
                tb = wp.tile([128, LG], f32, tag=f"tb{sfx}")
                nc.vector.tensor_scalar(
                    out=tb[:], in0=c_t[:], scalar1=-0.49999997, scalar2=None, op0=Alu.add
                )
                nc.vector.tensor_copy(out=fi_t[:], in_=tb[:])
                nc.vector.tensor_copy(out=f_t[:], in_=fi_t[:])
                nc.vector.tensor_tensor(out=w_t[:], in0=c_t[:], in1=f_t[:], op=Alu.subtract)

            # ---- span start index on GpSimd (int): fx*Hc + fy + base ----
            # host folds PAD + n*Hc*Wc - (SHIFT+4)*(Hc+1) into base
            hvi_b = AP(base_t[:], LG, [[1, NL], [0, GPP]])
            idxt = wp.tile([128, LG], i32, tag="idxt")
            nc.gpsimd.tensor_tensor(out=idxt[:], in0=fxi[:], in1=hvi_b, op=Alu.mult)
            nc.gpsimd.tensor_tensor(out=idxt[:], in0=idxt[:], in1=fyi[:], op=Alu.add)
            idx = wp.tile([128, LG], i32, tag="idx")
            nc.gpsimd.tensor_tensor(out=idx[:], in0=idxt[:], in1=base_t[:, :LG], op=Alu.add)

            # ---- gathers: per-wave indirect DMA (ucode: 1 offset/partition) ----
            # smallest level first: light transfers keep the SWDGE ring from
            # backlogging while mixes spin up
            patches = {}
            for l in (3, 2, 1, 0):
                Hc, Wc = LV[l]
                span = 9 * Hc + 10
                patch = pp.tile([128, GPP * span], f16, tag=f"patch{l}")
                patches[l] = patch
                for w in range(GPP):
                    gi = nc.gpsimd.indirect_dma_start(
                        out=patch[:, w * span : (w + 1) * span],
                        out_offset=None,
                        in_=srcs[l][0 if w < GPP // 2 else 1],
                        in_offset=bass.IndirectOffsetOnAxis(
                            ap=idx[:, l * GPP + w : l * GPP + w + 1], axis=0
                        ),
                    )
                    # pack the 128 per-partition descriptors into one SDMA
                    # packet (dma_gather's default); cuts per-gather overhead
                    gi.ins.single_packet = True

            # ---- masks & folded weights, fp16, level-stacked ----
            omx = wp.tile([128, LG], f32, tag="omx")
            omy = wp.tile([128, LG], f32, tag="omy")
            nc.vector.tensor_scalar(
                out=omx[:], in0=wx[:], scalar1=-1.0, scalar2=1.0, op0=Alu.mult, op1=Alu.add
            )
            nc.vector.tensor_scalar(
                out=omy[:], in0=wy[:], scalar1=-1.0, scalar2=1.0, op0=Alu.mult, op1=Alu.add
            )
            fx16 = wp.tile([128, LG], f16, tag="fx16")
            fy16 = wp.tile([128, LG], f16, tag="fy16")
            nc.vector.tensor_copy(out=fx16[:], in_=fx[:])
            nc.vector.tensor_copy(out=fy16[:], in_=fy[:])

            # xs[p,(l,g),r] = fx - SHIFT - 4 + r  (true x coord of column r)
            xs = wp.tile([128, LGR], f16, tag="xs")
            ys = wp.tile([128, LGR], f16, tag="ys")
            io_b = AP(io_t[:], 0, [[0, LG], [1, 10]])
            nc.vector.tensor_tensor(
                out=xs[:], in0=AP(fx16[:], 0, [[1, LG], [0, 10]]), in1=io_b, op=Alu.add
            )
            nc.vector.tensor_tensor(
                out=ys[:], in0=AP(fy16[:], 0, [[1, LG], [0, 10]]), in1=io_b, op=Alu.add
            )
            # clip to [0, Wc-1]/[0, Hc-1]; mask = (clipped == raw)
            xc = wp.tile([128, LGR], f16, tag="xc")
            yc = wp.tile([128, LGR], f16, tag="yc")
            nc.vector.tensor_tensor(out=xc[:], in0=xs[:], in1=ubx_t[:], op=Alu.min)
            nc.vector.tensor_tensor(out=yc[:], in0=ys[:], in1=uby_t[:], op=Alu.min)
            nc.vector.tensor_scalar(out=xc[:], in0=xc[:], scalar1=0.0, scalar2=None, op0=Alu.max)
            nc.vector.tensor_scalar(out=yc[:], in0=yc[:], scalar1=0.0, scalar2=None, op0=Alu.max)
            cmx = wp.tile([128, LGR], f16, tag="cmx")
            cmy = wp.tile([128, LGR], f16, tag="cmy")
            nc.vector.tensor_tensor(out=cmx[:], in0=xc[:], in1=xs[:], op=Alu.is_equal)
            nc.vector.tensor_tensor(out=cmy[:], in0=yc[:], in1=ys[:], op=Alu.is_equal)

            # broadcast-materialize the scalar weights to 9 taps (cast f32->f16)
            LG9 = LG * 9  # 576
            omyr = wp.tile([128, LG9], f16, tag="omyr")
            wyr = wp.tile([128, LG9], f16, tag="wyr")
            omxr = wp.tile([128, LG9], f16, tag="omxr")
            wxr = wp.tile([128, LG9], f16, tag="wxr")
            nc.vector.tensor_copy(out=omyr[:], in_=AP(omy[:], 0, [[1, LG], [0, 9]]))
            nc.vector.tensor_copy(out=wyr[:], in_=AP(wy[:], 0, [[1, LG], [0, 9]]))
            nc.vector.tensor_copy(out=omxr[:], in_=AP(omx[:], 0, [[1, LG], [0, 9]]))
            nc.vector.tensor_copy(out=wxr[:], in_=AP(wx[:], 0, [[1, LG], [0, 9]]))
            # w0[(l,g),b] = (1-wy)*cmy[b], w1 = wy*cmy[b+1] (b = y tap)
            # v0[(l,g),a] = (1-wx)*cmx[a], v1 = wx*cmx[a+1] (a = x tap)
            w0 = wp.tile([128, LG9], f16, tag="w0")
            w1 = wp.tile([128, LG9], f16, tag="w1")
            v0 = wp.tile([128, LG9], f16, tag="v0")
            v1 = wp.tile([128, LG9], f16, tag="v1")
            cmy0 = AP(cmy[:], 0, [[10, LG], [1, 9]])
            cmy1 = AP(cmy[:], 1, [[10, LG], [1, 9]])
            cmx0 = AP(cmx[:], 0, [[10, LG], [1, 9]])
            cmx1 = AP(cmx[:], 1, [[10, LG], [1, 9]])
            nc.vector.tensor_tensor(out=w0[:], in0=cmy0, in1=omyr[:], op=Alu.mult)
            nc.vector.tensor_tensor(out=w1[:], in0=cmy1, in1=wyr[:], op=Alu.mult)
            nc.vector.tensor_tensor(out=v0[:], in0=cmx0, in1=omxr[:], op=Alu.mult)
            nc.vector.tensor_tensor(out=v1[:], in0=cmx1, in1=wxr[:], op=Alu.mult)

            # ---- bilinear mix per level (fp16, packed operands) ----
            # L3..L1 whole; L0 in two wave-halves so the last mix chunk only
            # trails the final 8 gathers.
            def mix(l, w_lo, n_w, ot):
                Hc, Wc = LV[l]
                span = 9 * Hc + 10
                patch = patches[l]
                po = w_lo * span
                # P view [128, g, a(10 cols, stride Hc), b(9 rows, stride 1)]
                P0 = AP(patch[:], po, [[span, n_w], [Hc, 10], [1, 9]])
                P1 = AP(patch[:], po + 1, [[span, n_w], [Hc, 10], [1, 9]])
                t1 = wp.tile([128, n_w * 90], f16, tag=f"t1_{n_w}")
                t2 = wp.tile([128, n_w * 90], f16, tag=f"t2_{n_w}")
                t1v = AP(t1[:], 0, [[90, n_w], [9, 10], [1, 9]])
                t2v = AP(t2[:], 0, [[90, n_w], [9, 10], [1, 9]])
                wo = l * GPP * 9 + w_lo * 9
                w0b = AP(w0[:], wo, [[9, n_w], [0, 10], [1, 9]])
                w1b = AP(w1[:], wo, [[9, n_w], [0, 10], [1, 9]])
                # y-mix: t[g,a,b] = P[g,a,b]*w0[b] ; P[g,a,b+1]*w1[b]
                nc.vector.tensor_tensor(out=t1v, in0=P0, in1=w0b, op=Alu.mult)
                nc.vector.tensor_tensor(out=t2v, in0=P1, in1=w1b, op=Alu.mult)
                # qbT[g,b,a] = t1[g,a,b] + t2[g,a,b]  (transposed write)
                qb = wp.tile([128, n_w * 90], f16, tag=f"qb_{n_w}")
                qbT = AP(qb[:], 0, [[90, n_w], [10, 9], [1, 10]])
                t1t = AP(t1[:], 0, [[90, n_w], [1, 9], [9, 10]])
                t2t = AP(t2[:], 0, [[90, n_w], [1, 9], [9, 10]])
                nc.vector.tensor_tensor(out=qbT, in0=t1t, in1=t2t, op=Alu.add)
                # x-mix: out[g,b,a] = qb[g,b,a]*v0[a] + qb[g,b,a+1]*v1[a]
                u1 = wp.tile([128, n_w * 81], f16, tag=f"u1_{n_w}")
                u2 = wp.tile([128, n_w * 81], f16, tag=f"u2_{n_w}")
                Qa0 = AP(qb[:], 0, [[90, n_w], [10, 9], [1, 9]])
                Qa1 = AP(qb[:], 1, [[90, n_w], [10, 9], [1, 9]])
                u1v = AP(u1[:], 0, [[81, n_w], [9, 9], [1, 9]])
                u2v = AP(u2[:], 0, [[81, n_w], [9, 9], [1, 9]])
                v0b = AP(v0[:], wo, [[9, n_w], [0, 9], [1, 9]])
                v1b = AP(v1[:], wo, [[9, n_w], [0, 9], [1, 9]])
                nc.vector.tensor_tensor(out=u1v, in0=Qa0, in1=v0b, op=Alu.mult)
                nc.vector.tensor_tensor(out=u2v, in0=Qa1, in1=v1b, op=Alu.mult)
                oslice = ot[:, w_lo * 81 : (w_lo + n_w) * 81]
                nc.vector.tensor_tensor(out=oslice, in0=u1[:], in1=u2[:], op=Alu.add)

            for l in (3, 2, 1):
                ot = wp.tile([128, GPP * 81], f16, tag=f"ot{l}")
                mix(l, 0, GPP, ot)
                nc.sync.dma_start(out=outs[l], in_=ot[:])
            ot0 = wp.tile([128, GPP * 81], f16, tag="ot0")
            q = GPP // 4
            for h in range(4):
                mix(0, h * q, q, ot0)
                nc.sync.dma_start(
                    out=outs[0][:, h * q * 81 : (h + 1) * q * 81],
                    in_=ot0[:, h * q * 81 : (h + 1) * q * 81],
                )

    nc.compile()
    return nc


def _marshal(corr0, corr1, corr2, corr3, flow):
    """Build per-core input maps."""
    corrs = [corr0, corr1, corr2, corr3]
    fl = np.ascontiguousarray(flow.transpose(0, 2, 3, 1).reshape(N, 2))
    wgrid = np.tile(np.arange(W, dtype=np.float32), H * B)
    hgrid = np.tile(np.repeat(np.arange(H, dtype=np.float32), W), B)
    iota = np.tile(
        (np.arange(10, dtype=np.float16) - np.float16(4.0 + SHIFT)).reshape(1, 10),
        (128, 1),
    )
    svec = np.tile(
        np.array([1.0 / (1 << l) for l in range(NL)], dtype=np.float32), (128, 1)
    )
    # upper bounds per (l, g, r): Wc-1 / Hc-1 constant per level
    ubx = np.tile(
        np.repeat(
            np.array([LV[l][1] - 1 for l in range(NL)], dtype=np.float16), GPP * 10
        ).reshape(1, -1),
        (128, 1),
    )
    uby = np.tile(
        np.repeat(
            np.array([LV[l][0] - 1 for l in range(NL)], dtype=np.float16), GPP * 10
        ).reshape(1, -1),
        (128, 1),
    )
    # base[(p),(l,g)] = PAD + n_half*Hc*Wc - (SHIFT+4)*(Hc+1);
    # n_half = (g mod 8)*128 + p (waves 0-7 -> half a, 8-15 -> half b)
    p_ar = np.arange(128)[:, None, None]
    g_ar = np.arange(GPP)[None, None, :]
    base = np.empty((128, NL, GPP), dtype=np.int64)
    for l, (Hc, Wc) in enumerate(LV):
        base[:, l, :] = (
            PAD
            + ((g_ar % (GPP // 2)) * 128 + p_ar) * (Hc * Wc)
            - int((SHIFT + 4) * (Hc + 1))
        )[:, 0, :]
    base = np.concatenate(
        [
            base.reshape(128, NL * GPP),
            np.tile(np.array([LV[l][0] for l in range(NL)]), (128, 1)),
        ],
        axis=1,
    ).astype(np.int32)
    # per-level prescaled base coords, level-stacked (l, g)
    bxs = np.empty((128, NL, GPP), dtype=np.float32)
    bys = np.empty((128, NL, GPP), dtype=np.float32)

    in_maps = []
    for c in range(N_CORES):
        m = {}
        lo = c * NPX
        for l, (Hc, Wc) in enumerate(LV):
            shard = corrs[l].reshape(N, Hc, Wc)[lo : lo + NPX]
            tr = np.ascontiguousarray(shard.transpose(0, 2, 1)).reshape(NPX, -1)
            half = NPX // 2
            for h, sl in (("a", slice(0, half)), ("b", slice(half, NPX))):
                buf = np.zeros(half * Hc * Wc + 2 * PAD, dtype=np.float16)
                buf[PAD : PAD + half * Hc * Wc] = tr[sl].reshape(-1)
                m[f"src{l}{h}"] = buf.reshape(-1, 1)
        wm = lambda a: np.ascontiguousarray(a.reshape(GPP, 128).T)
        m["flxy"] = np.concatenate(
            [wm(fl[lo : lo + NPX, 0]), wm(fl[lo : lo + NPX, 1])], axis=1
        )
        bxw = wm(wgrid[lo : lo + NPX])
        byw = wm(hgrid[lo : lo + NPX])
        for l in range(NL):
            s = np.float32(1.0 / (1 << l))
            bxs[:, l, :] = bxw * s + np.float32(SHIFT)
            bys[:, l, :] = byw * s + np.float32(SHIFT)
        m["bxys"] = np.concatenate(
            [bxs.reshape(128, NL * GPP), bys.reshape(128, NL * GPP)], axis=1
        ).astype(np.float32)
        m["svec"] = svec
        m["base"] = base
        m["iot"] = iota
        m["ubx"] = ubx
        m["uby"] = uby
        in_maps.append(m)
    return in_maps


def kernel(corr0, corr1, corr2, corr3, flow):
    global _prog, LAST_EXEC_NS
    trace = os.environ.get("CORR_TRACE") == "1"
    if trace:
        trace = _install_trace_shim()
    from concourse.bass_utils import run_bass_kernel_spmd

    if _prog is None:
        _prog = _build()
    in_maps = _marshal(corr0, corr1, corr2, corr3, flow)
    res = run_bass_kernel_spmd(
        _prog,
        in_maps,
        core_ids=list(range(N_CORES)),
        trace=trace,
        trace_cores=[0] if trace else None,
    )
    LAST_EXEC_NS = res.exec_time_ns
    if trace and res.instructions_and_trace:
        kernel.last_insts = res.instructions_and_trace
    # assemble: out[n, l*81 + a*9 + b]; device gives [p, g, b, a]
    full = np.empty((N, 324), dtype=np.float32)
    for c in range(N_CORES):
        lo = c * NPX
        for l in range(4):
            o = res.results[c][f"out{l}"].reshape(128, GPP, 9, 9).astype(np.float32)
            full[lo : lo + NPX, l * 81 : (l + 1) * 81] = (
                o.transpose(1, 0, 3, 2).reshape(NPX, 81)
            )
    return np.ascontiguousarray(
        full.reshape(B, H, W, 324).transpose(0, 3, 1, 2)
    )
